# revision 8
# baseline (speedup 1.0000x reference)
"""Cross-attention (GQA, key-padding + shifted-causal mask) on 8 Trainium2 cores.

Sharding: core k handles batch b = k//4 and kv heads {2*(k%4), 2*(k%4)+1}
(each with its 4 query heads under GQA) -> 8 (b,h) attention instances per
core, fully independent (no collectives).

Mask algebra: the reference adds -10000 for padded keys and *replaces* with
-10000 where s > t + len_b - Sk. Since len_b >= Sk/2, the causal condition
subsumes the padding one, so the effective rule is "key s visible to query t
iff s <= t - c_b" with c_b = Sk - len_b. Rolling K/V right by c_b turns this
into a standard causal mask (s' <= t). Rolled-in garbage rows (s' < c_b) have
K=0 so their score is exactly 0 and exp gives exactly 1.0: they contribute 0
to the numerator (V rows zeroed) and exactly c_b to the plain denominator,
which the kernel subtracts back out (den = colsum(P) - c_b). Rows t < c_b
attend to nothing; the reference gives them mean(V) -- patched on host.

Per (b,h) the device computes, in score-transposed (ST) layout [s, t]:
  ST = K' @ Q^T            (f32r matmuls; causal blocks only, with the
                            diagonal band computed as column-trapezoids)
  P  = exp(scale * ST)     (ScalarE)
  P *= diag_mask           (gpsimd affine_select, 128-col corners only)
  OT   = sum_s V'[s,d] P[s,t]   (PSUM accum over s blocks)
  acc  = sum_blocks P           (DVE/Pool elementwise accumulation)
  den  = ones^T @ acc - c_b     (one matmul per t-tile + Pool subtract)
  out  = OT * 1/den             (VectorE), stored d-major; host transposes
                                 back to (B, Sq, H, D).
"""

import numpy as np

B, SQ, SK, H, HK, D = 2, 2048, 2048, 32, 8, 128
G = H // HK            # query heads per kv head
N_CORES = 8
TQ = 512               # t (query) tile width
TS = 128               # s (key) tile width
NTQ = SQ // TQ         # 4 t-chunks
SCALE = 1.0 / float(np.sqrt(D))

_compiled = None


def _build_program():
    """Build + schedule the single SPMD Bass program (same for all cores)."""
    from contextlib import ExitStack
    import concourse.bass as bass
    import concourse.tile as tile
    from concourse import bacc, mybir

    f32 = mybir.dt.float32
    f32r = mybir.dt.float32r

    nc = bacc.Bacc("TRN2", target_bir_lowering=False, debug=False)
    qT_ap = nc.dram_tensor("qT", [2 * G, D, SQ], f32, kind="ExternalInput").ap()
    kT_ap = nc.dram_tensor("kT", [2, D, SK], f32, kind="ExternalInput").ap()
    v_ap = nc.dram_tensor("v", [2, TS, SK // TS * D], f32, kind="ExternalInput").ap()
    cbn_ap = nc.dram_tensor("cbn", [TS, 1], f32, kind="ExternalInput").ap()
    out_ap = nc.dram_tensor("out", [2 * G, D, SQ], f32, kind="ExternalOutput").ap()

    with tile.TileContext(nc) as tc, ExitStack() as ctx:
        const_pool = ctx.enter_context(tc.tile_pool(name="const", bufs=1))
        kv_pool = ctx.enter_context(tc.tile_pool(name="kv", bufs=2))
        q_pool = ctx.enter_context(tc.tile_pool(name="q", bufs=2))
        p_pool = ctx.enter_context(tc.tile_pool(name="p", bufs=6))
        acc_pool = ctx.enter_context(tc.tile_pool(name="acc", bufs=3))
        fin_pool = ctx.enter_context(tc.tile_pool(name="fin", bufs=3))
        st_psum = ctx.enter_context(tc.tile_pool(name="st", bufs=2, space="PSUM"))
        ot_psum = ctx.enter_context(tc.tile_pool(name="ot", bufs=3, space="PSUM"))
        den_psum = ctx.enter_context(tc.tile_pool(name="den", bufs=1, space="PSUM"))

        ones_sb = const_pool.tile([TS, TS], f32)
        nc.vector.memset(ones_sb[:], 1.0)
        cbn_sb = const_pool.tile([TS, 1], f32)
        nc.sync.dma_start(cbn_sb[:], cbn_ap[:])

        # deferred finalization state: (last_pv_closure, ih, t, acc, ot_ps)
        deferred = [None]

        def finalize():
            if deferred[0] is None:
                return
            last_pv, ih, t, acc, ot_ps = deferred[0]
            deferred[0] = None
            last_pv()
            den_ps = den_psum.tile([TS, TQ], f32)
            nc.tensor.matmul(
                den_ps[:], lhsT=ones_sb[:].bitcast(f32r), rhs=acc[:],
                start=True, stop=True,
            )
            den2_sb = fin_pool.tile([TS, TQ], f32, tag="den2")
            nc.vector.tensor_scalar_add(den2_sb[:], den_ps[:], cbn_sb[:, 0:1])
            recip_sb = fin_pool.tile([TS, TQ], f32, tag="recip")
            nc.vector.reciprocal_approx_fast(recip_sb[:], den2_sb[:])
            out_sb = fin_pool.tile([D, TQ], f32, tag="out")
            nc.vector.tensor_tensor(
                out=out_sb[:], in0=ot_ps[:], in1=recip_sb[:],
                op=mybir.AluOpType.mult,
            )
            nc.sync.dma_start(out_ap[ih][:, t * TQ : (t + 1) * TQ], out_sb[:])

        for ikv in range(2):
            kT_sb = kv_pool.tile([D, SK], f32r, tag="kT")
            v_sb = kv_pool.tile([TS, SK // TS * D], f32r, tag="v")
            if ikv == 0:
                # startup: order sync queue by first consumption, park the
                # not-immediately-needed loads on the idle gpsimd queue
                nc.sync.dma_start(kT_sb[:, :TQ], kT_ap[ikv][:, :TQ].bitcast(f32r))
                nc.gpsimd.dma_start(v_sb[:], v_ap[ikv].bitcast(f32r))
            else:
                nc.sync.dma_start(kT_sb[:], kT_ap[ikv].bitcast(f32r))
                nc.sync.dma_start(v_sb[:], v_ap[ikv].bitcast(f32r))

            for j in range(G):
                ih = ikv * G + j
                qT_sb = q_pool.tile([D, SQ], f32r)
                if ikv == 0 and j == 0:
                    nc.sync.dma_start(qT_sb[:, :TQ], qT_ap[ih][:, :TQ].bitcast(f32r))
                    nc.sync.dma_start(qT_sb[:, TQ:], qT_ap[ih][:, TQ:].bitcast(f32r))
                    nc.sync.dma_start(kT_sb[:, TQ:], kT_ap[ikv][:, TQ:].bitcast(f32r))
                else:
                    nc.sync.dma_start(qT_sb[:], qT_ap[ih].bitcast(f32r))

                for t in range(NTQ):
                    m = 4 * t          # off-diagonal s blocks
                    tq0 = t * TQ
                    qs = qT_sb[:, tq0 : tq0 + TQ]
                    ot_ps = ot_psum.tile([D, TQ], f32)
                    acc = acc_pool.tile([TS, TQ], f32r)
                    eng = nc.vector if t in (0, 2) else nc.gpsimd

                    def add_acc(dst, src):
                        eng.tensor_tensor(
                            out=dst, in0=dst, in1=src, op=mybir.AluOpType.add
                        )

                    # groups: ("pair", pi) | ("diagA",) | ("diagB",)
                    groups = [("pair", pi) for pi in range(m // 2)]
                    groups += [("diagA",), ("diagB",)]
                    pending = None  # (kind, p tile) for 1-deep PE pipeline

                    def emit_pv(pend, ot_ps=ot_ps, v_sb=v_sb, m=m):
                        kind, p = pend[0], pend[1]
                        if kind == "pair":
                            sc0 = pend[2]
                            for h in range(2):
                                nc.tensor.matmul(
                                    ot_ps[:],
                                    lhsT=v_sb[:, (sc0 + h) * D : (sc0 + h + 1) * D],
                                    rhs=p[:, h * TQ : (h + 1) * TQ],
                                    start=(sc0 == 0 and h == 0), stop=False,
                                )
                        elif kind == "diagA":
                            nc.tensor.matmul(
                                ot_ps[:],
                                lhsT=v_sb[:, m * D : (m + 1) * D],
                                rhs=p[:, 0:TQ],
                                start=(m == 0), stop=False,
                            )
                            nc.tensor.matmul(
                                ot_ps[:, TS:TQ],
                                lhsT=v_sb[:, (m + 1) * D : (m + 2) * D],
                                rhs=p[:, TQ : TQ + 384],
                                start=False, stop=False, skip_group_check=True,
                            )
                        else:  # diagB
                            nc.tensor.matmul(
                                ot_ps[:, 2 * TS : TQ],
                                lhsT=v_sb[:, (m + 2) * D : (m + 3) * D],
                                rhs=p[:, 0:256],
                                start=False, stop=False, skip_group_check=True,
                            )
                            nc.tensor.matmul(
                                ot_ps[:, 2 * TS : TQ],
                                lhsT=v_sb[:, (m + 3) * D : (m + 4) * D],
                                rhs=p[:, 256:512],
                                start=False, stop=True, skip_group_check=True,
                            )

                    for gi, g in enumerate(groups):
                        st = st_psum.tile([TS, 2 * TQ], f32)
                        p = p_pool.tile([TS, 2 * TQ], f32r)
                        if g[0] == "pair":
                            sc0 = 2 * g[1]
                            for h in range(2):
                                nc.tensor.matmul(
                                    st[:, h * TQ : (h + 1) * TQ],
                                    lhsT=kT_sb[:, (sc0 + h) * TS : (sc0 + h + 1) * TS],
                                    rhs=qs, start=True, stop=True,
                                )
                            if gi == 1:
                                finalize()  # prev tile's den/recip/mult/dma
                            nc.scalar.activation(
                                p[:], st[:],
                                mybir.ActivationFunctionType.Exp, scale=SCALE,
                            )
                            if pending is not None:
                                emit_pv(pending)
                            if gi == 0:
                                eng.tensor_tensor(
                                    out=acc[:], in0=p[:, 0:TQ], in1=p[:, TQ : 2 * TQ],
                                    op=mybir.AluOpType.add,
                                )
                            else:
                                add_acc(acc[:], p[:, 0:TQ])
                                add_acc(acc[:], p[:, TQ : 2 * TQ])
                            pending = ("pair", p, sc0)
                        elif g[0] == "diagA":
                            # block m: full 512 cols; block m+1: cols 128..512
                            nc.tensor.matmul(
                                st[:, 0:TQ],
                                lhsT=kT_sb[:, m * TS : (m + 1) * TS],
                                rhs=qs, start=True, stop=True,
                            )
                            nc.tensor.matmul(
                                st[:, TQ : TQ + 384],
                                lhsT=kT_sb[:, (m + 1) * TS : (m + 2) * TS],
                                rhs=qT_sb[:, tq0 + TS : tq0 + TQ],
                                start=True, stop=True,
                            )
                            if gi == 1:
                                finalize()
                            nc.scalar.activation(
                                p[:, 0 : TQ + 384], st[:, 0 : TQ + 384],
                                mybir.ActivationFunctionType.Exp, scale=SCALE,
                            )
                            # causal corners: keep j >= partition
                            for off in (0, TQ):
                                nc.gpsimd.affine_select(
                                    out=p[:, off : off + TS],
                                    in_=p[:, off : off + TS],
                                    pattern=[[1, TS]],
                                    compare_op=mybir.AluOpType.is_ge,
                                    fill=0.0, base=0, channel_multiplier=-1,
                                )
                            if pending is not None:
                                emit_pv(pending)
                            if m == 0:
                                eng.tensor_scalar_add(acc[:], p[:, 0:TQ], 0.0)
                            else:
                                add_acc(acc[:], p[:, 0:TQ])
                            add_acc(acc[:, TS:TQ], p[:, TQ : TQ + 384])
                            pending = ("diagA", p)
                        else:  # diagB: blocks m+2 (cols 256:512), m+3 (widened)
                            nc.tensor.matmul(
                                st[:, 0:256],
                                lhsT=kT_sb[:, (m + 2) * TS : (m + 3) * TS],
                                rhs=qT_sb[:, tq0 + 256 : tq0 + TQ],
                                start=True, stop=True,
                            )
                            nc.tensor.matmul(
                                st[:, 256:512],
                                lhsT=kT_sb[:, (m + 3) * TS : (m + 4) * TS],
                                rhs=qT_sb[:, tq0 + 256 : tq0 + TQ],
                                start=True, stop=True,
                            )
                            if gi == 1:
                                finalize()
                            nc.scalar.activation(
                                p[:, 0:TQ], st[:, 0:TQ],
                                mybir.ActivationFunctionType.Exp, scale=SCALE,
                            )
                            nc.gpsimd.affine_select(
                                out=p[:, 0:TS], in_=p[:, 0:TS],
                                pattern=[[1, TS]],
                                compare_op=mybir.AluOpType.is_ge,
                                fill=0.0, base=0, channel_multiplier=-1,
                            )
                            nc.gpsimd.affine_select(
                                out=p[:, 256:512], in_=p[:, 256:512],
                                pattern=[[1, 256]],
                                compare_op=mybir.AluOpType.is_ge,
                                fill=0.0, base=-TS, channel_multiplier=-1,
                            )
                            if pending is not None:
                                emit_pv(pending)
                            add_acc(acc[:, 256:TQ], p[:, 0:256])
                            add_acc(acc[:, 256:TQ], p[:, 256:512])
                            pending = ("diagB", p)
                    # last PV + den/recip/mult/dma deferred into the next tile
                    deferred[0] = (
                        (lambda pend=pending, epv=emit_pv: epv(pend)),
                        ih, t, acc, ot_ps,
                    )
        finalize()

    nc.compile()
    return nc


def _get_program():
    global _compiled
    if _compiled is None:
        _compiled = _build_program()
    return _compiled


def kernel(q, kv, key_padding_mask, _want_trace=False):
    q = np.asarray(q, dtype=np.float32)
    kv = np.asarray(kv, dtype=np.float32)
    mask = np.asarray(key_padding_mask).astype(bool)

    lengths = mask.sum(axis=1).astype(np.int64)  # valid keys per batch
    c = SK - lengths                             # roll shift per batch

    k_full = kv[:, :, 0]  # (B, SK, HK, D)
    v_full = kv[:, :, 1]

    # roll keys/values right by c[b]; only the first len_b keys are ever
    # visible so the tail [len_b:] is dropped. Pad region stays zero.
    k_roll = np.zeros_like(k_full)
    v_roll = np.zeros_like(v_full)
    for b in range(B):
        k_roll[b, c[b]:] = k_full[b, : lengths[b]]
        v_roll[b, c[b]:] = v_full[b, : lengths[b]]

    in_maps = []
    for core in range(N_CORES):
        b = core // 4
        hks = (2 * (core % 4), 2 * (core % 4) + 1)
        qT = np.empty((2 * G, D, SQ), dtype=np.float32)
        kT = np.empty((2, D, SK), dtype=np.float32)
        v_l = np.empty((2, TS, SK // TS * D), dtype=np.float32)
        for i, hk in enumerate(hks):
            kT[i] = k_roll[b, :, hk, :].T
            # v chunked: v_l[i][p, sc*D + d] = v_roll[b, sc*TS + p, hk, d]
            v_l[i] = np.ascontiguousarray(
                v_roll[b, :, hk, :].reshape(SK // TS, TS, D).transpose(1, 0, 2)
            ).reshape(TS, SK // TS * D)
            for j in range(G):
                qT[i * G + j] = q[b, :, hk * G + j, :].T
        cbn = np.full((TS, 1), -float(c[b]), dtype=np.float32)
        in_maps.append({
            "qT": np.ascontiguousarray(qT),
            "kT": np.ascontiguousarray(kT),
            "v": np.ascontiguousarray(v_l),
            "cbn": cbn,
        })

    from concourse.bass_utils import run_bass_kernel_spmd

    nc = _get_program()
    res = run_bass_kernel_spmd(
        nc, in_maps, core_ids=list(range(N_CORES)),
        trace=_want_trace,
    )

    out = np.empty((B, SQ, H, D), dtype=np.float32)
    for core in range(N_CORES):
        b = core // 4
        hks = (2 * (core % 4), 2 * (core % 4) + 1)
        o_core = res.results[core]["out"]  # (2*G, D, SQ)
        for i, hk in enumerate(hks):
            for j in range(G):
                out[b, :, hk * G + j, :] = o_core[i * G + j].T

    # rows that attend to nothing: reference softmax is uniform -> mean(V)
    for b in range(B):
        if c[b] > 0:
            vm = v_full[b].mean(axis=0)  # (HK, D)
            out[b, : c[b]] = np.repeat(vm, G, axis=0)[None]

    if _want_trace:
        return out, res
    return out
